# revision 1
# baseline (speedup 1.0000x reference)
"""Low-rank (random-feature) attention kernel for Trainium2, 8 NeuronCores — v2.

Sharding: flatten hidden_states to [B*S, H] = [32768, 768] rows; core c owns
4096 contiguous rows (= batch c//2, sequence half c%2).  The per-(batch, head)
kv summary is reduced with two pipelined pairwise AllReduces (~200 KB each).

v2 structural changes vs v1:
- x is transposed on the HOST: the device loads xT [H, R] with plain
  contiguous DMAs (v1 spent ~40 us in serialized DMA_TRANSPOSEs before the
  first matmul).
- The feature projection P is folded into Wk on the host (like Wq), so the
  k-side is a single GEMM producing kp row-major; the separate on-device
  per-pair projection stage is gone.
- The per-(row, head) k-max is applied by scaling V rows with exp(-max)
  (exactly equivalent because kv/ksum only combine kp and v within a head),
  turning 12 small per-head exps per row tile into one reduce + 2 exps + one
  broadcast multiply.
- Biases: bq/bk fold into the exps (host-folded); bv/bk-exp factors are
  applied as a cheap rank-1/diagonal fixup on the 780-element kv summary
  after the collective; bo rides the output-copy add.
- The normalizer reciprocal is broadcast across partitions with a K=2
  selector matmul into PSUM and fused into the ctx PSUM->SBUF copy.
- ~40 warm-up matmuls on junk data keep the PE HAM clock-gate at full rate
  through the initial DMA phase (v1 ran ~43% of its matmul time at 1.2 GHz).
- bf16 everywhere off-chip (including the output, cast back on host).
"""

import sys

sys.path.insert(0, "/opt/trn_rl_repo")

import contextlib

import ml_dtypes
import numpy as np

import concourse.bass as bass
import concourse.tile as tile
from concourse import mybir
from concourse.bass_utils import run_bass_kernel_spmd

BF16 = mybir.dt.bfloat16
F32 = mybir.dt.float32
AF = mybir.ActivationFunctionType
ALU = mybir.AluOpType
AX = mybir.AxisListType

B, S, H = 4, 8192, 768
NH, HD, M = 12, 64, 64
EPS = 1e-6
NCORES = 8
R = (B * S) // NCORES          # rows per core = 4096
NPAIR = NH // 2                # head pairs = 6
KT = H // 128                  # hidden k-tiles = 6
RT = 128                       # row tile
NRT = R // RT                  # 32 row tiles
CHUNK = 512
NCH = R // CHUNK               # 8 chunks
WARM_MM = 44                   # PE warm-up matmuls during the load phase


def _split_multi_waits(nc):
    """This container's walrus only accepts one semaphore wait per
    instruction; hoist extra waits onto same-engine NoOps placed before."""
    c = 0
    for f in nc.m.functions:
        for bb in f.blocks:
            new_insts = []
            for ins in bb.instructions:
                si = ins.sync_info
                if si is not None and si.on_wait and len(si.on_wait) > 1:
                    waits = list(si.on_wait)
                    for w in waits[:-1]:
                        c += 1
                        new_insts.append(mybir.InstNoOp(
                            name=f"I-waitsplit-{c}", engine=ins.engine,
                            sync_info=mybir.SyncInfo(on_wait=[w], on_update=[])))
                    ins.sync_info = mybir.SyncInfo(
                        on_wait=[waits[-1]], on_update=list(si.on_update))
                new_insts.append(ins)
            bb.instructions[:] = new_insts
    return c


def _bc_free(ap, n):
    """Broadcast an AP along a new innermost (stride-0) free axis of size n."""
    return bass.AP(tensor=ap.tensor, offset=ap.offset, ap=list(ap.ap) + [[0, n]])


def _build(nc):
    xt = nc.dram_tensor("xt", [H, R], BF16, kind="ExternalInput")
    wkp = nc.dram_tensor("wkp", [H, H], BF16, kind="ExternalInput")
    wv = nc.dram_tensor("wv", [H, H], BF16, kind="ExternalInput")
    wqp = nc.dram_tensor("wqp", [H, H], BF16, kind="ExternalInput")
    wo = nc.dram_tensor("wo", [H, H], BF16, kind="ExternalInput")
    bqpc_d = nc.dram_tensor("bqpc", [128, KT], F32, kind="ExternalInput")
    ebkp_d = nc.dram_tensor("ebkp", [128, NPAIR], F32, kind="ExternalInput")
    bvbc_d = nc.dram_tensor("bvbc", [128, NPAIR, 130], F32, kind="ExternalInput")
    bobc_d = nc.dram_tensor("bobc", [128, H], F32, kind="ExternalInput")
    sel12_d = nc.dram_tensor("sel12", [NH, NPAIR, 128], BF16, kind="ExternalInput")
    out = nc.dram_tensor("out", [R, H], BF16, kind="ExternalOutput")

    with tile.TileContext(nc) as tc, contextlib.ExitStack() as ctx:
        persist = ctx.enter_context(tc.tile_pool(name="persist", bufs=1))
        dram = ctx.enter_context(tc.tile_pool(name="dram", bufs=1, space="DRAM"))

        # ---- PE warm-up fodder: available immediately (no DMA dependency) ----
        junk = persist.tile([128, 512], BF16, tag="junk", name="junk")
        nc.vector.memset(junk[:], 1.0)
        epsc = persist.tile([128, 1], F32, tag="epsc", name="epsc")
        nc.vector.memset(epsc[:], EPS)

        # ---- constants (gpsimd SWDGE queue, small) ----
        bqpc = persist.tile([128, KT], F32, tag="bqpc", name="bqpc")
        nc.gpsimd.dma_start(out=bqpc[:], in_=bqpc_d.ap())
        ebkp = persist.tile([128, NPAIR], F32, tag="ebkp", name="ebkp")
        nc.gpsimd.dma_start(out=ebkp[:], in_=ebkp_d.ap())
        bvbc = persist.tile([128, NPAIR, 130], F32, tag="bvbc", name="bvbc")
        nc.gpsimd.dma_start(out=bvbc[:], in_=bvbc_d.ap())
        bobc = persist.tile([128, H], F32, tag="bobc", name="bobc")
        nc.gpsimd.dma_start(out=bobc[:], in_=bobc_d.ap())
        sel12 = persist.tile([NH, NPAIR, 128], BF16, tag="sel12", name="sel12")
        nc.gpsimd.dma_start(out=sel12[:], in_=sel12_d.ap())

        # ---- weights on the scalar HWDGE queue (parallel with x on sync) ----
        wkp_sb = persist.tile([128, KT, H], BF16, tag="wkp", name="wkp_sb")
        nc.scalar.dma_start(out=wkp_sb[:], in_=wkp.ap().rearrange("(k p) n -> p k n", p=128))
        wv_sb = persist.tile([128, KT, H], BF16, tag="wv", name="wv_sb")
        nc.scalar.dma_start(out=wv_sb[:], in_=wv.ap().rearrange("(k p) n -> p k n", p=128))
        wqp_sb = persist.tile([128, KT, H], BF16, tag="wqp", name="wqp_sb")
        nc.scalar.dma_start(out=wqp_sb[:], in_=wqp.ap().rearrange("(k p) n -> p k n", p=128))
        wo_sb = persist.tile([128, KT, H], BF16, tag="wo", name="wo_sb")
        nc.scalar.dma_start(out=wo_sb[:], in_=wo.ap().rearrange("(k p) n -> p k n", p=128))

        # ---- xT loads: 12 contiguous DMAs (kt x half) on the sync queue ----
        xt_sb = persist.tile([128, KT, R], BF16, tag="xt", name="xt_sb")
        for hh in range(2):
            hs = slice(hh * (R // 2), (hh + 1) * (R // 2))
            for kt in range(KT):
                nc.sync.dma_start(out=xt_sb[:, kt, hs],
                                  in_=xt[kt * 128:(kt + 1) * 128, hs])

        # ---- persistent result tiles ----
        qpT = [persist.tile([128, R], BF16, tag=f"qpT{p}", name=f"qpT{p}")
               for p in range(NPAIR)]
        kv_acc = [persist.tile([128, NPAIR, 130], F32, tag=f"kvacc{h}",
                               name=f"kv_acc{h}") for h in range(2)]
        kv_fix = persist.tile([128, NPAIR, 130], F32, tag="kvfix", name="kv_fix")
        bd_kv = persist.tile([128, NPAIR, 128], BF16, tag="bdkv", name="bd_kv")
        ks_bd = persist.tile([128, NPAIR, NH], BF16, tag="ksbd", name="ks_bd")
        r_cat = persist.tile([NH, R], BF16, tag="rcat", name="r_cat")

        cc_in = [dram.tile([128, NPAIR * 130], F32, name=f"cc_in{h}") for h in range(2)]
        cc_out = [dram.tile([128, NPAIR * 130], F32, name=f"cc_out{h}") for h in range(2)]

        # ================= Phase 1: k/v pass + kv accumulation =================
        with contextlib.ExitStack() as kctx:
            kvsb = kctx.enter_context(tc.tile_pool(name="kvsb", bufs=2))
            kvps = kctx.enter_context(tc.tile_pool(name="kvps", bufs=1, space="PSUM"))

            # warm-up matmuls share the kp_ps PSUM slot (bufs=1)
            warm_ps = kvps.tile([128, H], F32, tag="kp_ps", name="warm_ps")
            for i in range(WARM_MM):
                nc.tensor.matmul(warm_ps[:, 0:512], junk[:, 0:128], junk[:, :],
                                 start=True, stop=True)

            kv_ps = [None, None]   # [kv_a, kv_b] of current half
            kp_sbs = {}
            v_sbs = {}

            def emit_kv(rt):
                hh, rl = rt // (NRT // 2), rt % (NRT // 2)
                if rl == 0:
                    kv_ps[0] = kvps.tile([128, 3, 130], F32, tag="kv_a",
                                         bufs=2, name=f"kv_a{hh}")
                    kv_ps[1] = kvps.tile([128, 3, 130], F32, tag="kv_b",
                                         bufs=2, name=f"kv_b{hh}")
                kp_sb, v_sb = kp_sbs.pop(rt), v_sbs.pop(rt)
                for p in range(NPAIR):
                    # start=True clears has_written for the WHOLE bank, so only
                    # the first region of each bank may issue it; the other two
                    # pairs overwrite-on-clear at rl==0 via has_written=0.
                    nc.tensor.matmul(
                        kv_ps[p // 3][:, p % 3, :],
                        kp_sb[:, p * 128:(p + 1) * 128],
                        v_sb[:, 2 * p:2 * p + 2, :],
                        start=(rl == 0 and p % 3 == 0),
                        stop=(rl == NRT // 2 - 1),
                        skip_group_check=True)

            def drain_kv(hh):
                nc.vector.tensor_copy(out=kv_acc[hh][:, 0:3, :], in_=kv_ps[0][:])
                nc.vector.tensor_copy(out=kv_acc[hh][:, 3:6, :], in_=kv_ps[1][:])
                nc.sync.dma_start(out=cc_in[hh][:],
                                  in_=kv_acc[hh].rearrange("p a b -> p (a b)"))
                nc.gpsimd.collective_compute(
                    "AllReduce", ALU.add,
                    replica_groups=[[0, 1], [2, 3], [4, 5], [6, 7]],
                    ins=[cc_in[hh].opt()], outs=[cc_out[hh].opt()])

            for rt in range(NRT):
                rs = slice(rt * RT, (rt + 1) * RT)
                kp_ps = kvps.tile([128, H], F32, tag="kp_ps", name=f"kp_ps{rt}")
                v_ps = kvps.tile([128, H], F32, tag="v_ps", name=f"v_ps{rt}")
                for kt in range(KT):
                    xblk = xt_sb[:, kt, rs]
                    st, sp = (kt == 0), (kt == KT - 1)
                    nc.tensor.matmul(kp_ps[:, 0:512], xblk, wkp_sb[:, kt, 0:512],
                                     start=st, stop=sp)
                    nc.tensor.matmul(kp_ps[:, 512:768], xblk, wkp_sb[:, kt, 512:768],
                                     start=st, stop=sp)
                    nc.tensor.matmul(v_ps[:, 0:512], xblk, wv_sb[:, kt, 0:512],
                                     start=st, stop=sp)
                    nc.tensor.matmul(v_ps[:, 512:768], xblk, wv_sb[:, kt, 512:768],
                                     start=st, stop=sp)
                # kv outer products for the previous row tile (pipelined so the
                # exp/scale chain has a full GEMM's worth of slack)
                if rt > 0:
                    emit_kv(rt - 1)
                    if rt == NRT // 2:
                        drain_kv(0)
                # postprocess this row tile
                nmx = kvsb.tile([128, NH], F32, tag="nmx", name=f"nmx{rt}")
                nc.vector.tensor_reduce(
                    out=nmx[:], in_=kp_ps.rearrange("p (h m) -> p h m", m=M),
                    op=ALU.max, axis=AX.X, negate=True)
                emax = kvsb.tile([128, NH], F32, tag="emax", name=f"emax{rt}")
                nc.scalar.activation(out=emax[:], in_=nmx[:], func=AF.Exp)
                kp_sb = kvsb.tile([128, H], BF16, tag="kp_sb", name=f"kp_sb{rt}")
                nc.scalar.activation(out=kp_sb[:], in_=kp_ps[:], func=AF.Exp)
                v_sb = kvsb.tile([128, NH, 65], BF16, tag="v_sb", name=f"v_sb{rt}")
                nc.vector.tensor_tensor(
                    out=v_sb[:, :, 0:64],
                    in0=v_ps.rearrange("p (h d) -> p h d", d=HD),
                    in1=_bc_free(emax[:], HD), op=ALU.mult)
                nc.vector.tensor_copy(out=v_sb[:, :, 64:65], in_=emax[:])
                kp_sbs[rt], v_sbs[rt] = kp_sb, v_sb
            emit_kv(NRT - 1)
            drain_kv(1)

        # ---- collective readback + kv fixups (overlap the q pass on DVE) ----
        kv_r = [persist.tile([128, NPAIR, 130], F32, tag=f"kvr{h}", name=f"kv_r{h}")
                for h in range(2)]
        for hh in range(2):
            nc.sync.dma_start(out=kv_r[hh][:],
                              in_=cc_out[hh].rearrange("p (a b) -> p a b", b=130))
        nc.vector.tensor_add(out=kv_fix[:], in0=kv_r[0][:], in1=kv_r[1][:])
        # kv_true = ebkp * (kv0 + ksum0 (x) bv); ksum col itself has bvbc == 0
        for p in range(NPAIR):
            for half, col in ((slice(0, 64), 64), (slice(64, 128), 129)):
                nc.vector.scalar_tensor_tensor(
                    out=kv_fix[half, p, :], in0=bvbc[half, p, :],
                    scalar=kv_fix[half, p, col:col + 1], in1=kv_fix[half, p, :],
                    op0=ALU.mult, op1=ALU.add)
            nc.vector.tensor_scalar_mul(out=kv_fix[:, p, :], in0=kv_fix[:, p, :],
                                        scalar1=ebkp[:, p:p + 1])
        # block-diagonal kv + ksum for the ctx / normalizer matmuls
        nc.vector.memset(bd_kv[:], 0.0)
        nc.vector.memset(ks_bd[:], 0.0)
        for p in range(NPAIR):
            nc.vector.tensor_copy(out=bd_kv[0:64, p, 0:64], in_=kv_fix[0:64, p, 0:64])
            nc.vector.tensor_copy(out=bd_kv[64:128, p, 64:128],
                                  in_=kv_fix[64:128, p, 65:129])
            nc.vector.tensor_copy(out=ks_bd[0:64, p, 2 * p:2 * p + 1],
                                  in_=kv_fix[0:64, p, 64:65])
            nc.vector.tensor_copy(out=ks_bd[64:128, p, 2 * p + 1:2 * p + 2],
                                  in_=kv_fix[64:128, p, 129:130])

        # ================= Phase 2: q pass, then normalizer =================
        with contextlib.ExitStack() as qctx:
            qsb = qctx.enter_context(tc.tile_pool(name="qsb", bufs=2))
            qps = qctx.enter_context(tc.tile_pool(name="qps", bufs=1, space="PSUM"))
            for ch in range(NCH):
                cs = slice(ch * CHUNK, (ch + 1) * CHUNK)
                for ct in range(KT):
                    qp_ps = qps.tile([128, CHUNK], F32, tag="qp", bufs=3,
                                     name=f"qp_ps{ch}_{ct}")
                    for kt in range(KT):
                        nc.tensor.matmul(qp_ps[:],
                                         wqp_sb[:, kt, ct * 128:(ct + 1) * 128],
                                         xt_sb[:, kt, cs],
                                         start=(kt == 0), stop=(kt == KT - 1))
                    nc.scalar.activation(out=qpT[ct][:, cs], in_=qp_ps[:],
                                         func=AF.Exp, bias=bqpc[:, ct:ct + 1])
            # normalizer: PSUM drains on ACT (frees the np banks quickly, no
            # DVE-order entanglement), then the slow DVE reciprocals pipelined
            # ahead of phase 3's per-chunk consumption
            n_eps_t = []
            for ch in range(NCH):
                cs = slice(ch * CHUNK, (ch + 1) * CHUNK)
                n_ps = qps.tile([NH, CHUNK], F32, tag="np", bufs=2, name=f"n_ps{ch}")
                for p in range(NPAIR):
                    nc.tensor.matmul(n_ps[:], ks_bd[:, p, :], qpT[p][:, cs],
                                     start=(p == 0), stop=(p == NPAIR - 1))
                n_eps = qsb.tile([NH, CHUNK], F32, tag="neps", bufs=4,
                                 name=f"n_eps{ch}")
                nc.scalar.activation(out=n_eps[:], in_=n_ps[:], func=AF.Identity,
                                     bias=epsc[0:NH, :])
                n_eps_t.append(n_eps)
            for ch in range(NCH):
                cs = slice(ch * CHUNK, (ch + 1) * CHUNK)
                r_f32 = qsb.tile([NH, CHUNK], F32, tag="rf32", name=f"r_f32{ch}")
                nc.vector.reciprocal(out=r_f32[:], in_=n_eps_t[ch][:])
                nc.vector.tensor_copy(out=r_cat[:, cs], in_=r_f32[:])

        # ================= Phase 3: ctx + output projection =================
        with contextlib.ExitStack() as cctx:
            csb = cctx.enter_context(tc.tile_pool(name="csb", bufs=2))
            cps = cctx.enter_context(tc.tile_pool(name="cps", bufs=1, space="PSUM"))
            for ch in range(NCH):
                cs = slice(ch * CHUNK, (ch + 1) * CHUNK)
                ctx_ch = csb.tile([128, NPAIR, CHUNK], BF16, tag="ctx",
                                  name=f"ctx{ch}")
                for p in range(NPAIR):
                    rb_ps = cps.tile([128, CHUNK], F32, tag="rb", bufs=2,
                                     name=f"rb_ps{ch}_{p}")
                    nc.tensor.matmul(rb_ps[:], sel12[:, p, :], r_cat[:, cs],
                                     start=True, stop=True)
                    rb_sb = csb.tile([128, CHUNK], F32, tag="rbsb", bufs=2,
                                     name=f"rb_sb{ch}_{p}")
                    nc.scalar.activation(out=rb_sb[:], in_=rb_ps[:], func=AF.Copy)
                    a_ps = cps.tile([128, CHUNK], F32, tag="a", bufs=2,
                                    name=f"a_ps{ch}_{p}")
                    nc.tensor.matmul(a_ps[:], bd_kv[:, p, :], qpT[p][:, cs],
                                     start=True, stop=True)
                    nc.vector.tensor_tensor(out=ctx_ch[:, p, :], in0=a_ps[:],
                                            in1=rb_sb[:], op=ALU.mult)
                for r4 in range(CHUNK // RT):
                    rt = ch * (CHUNK // RT) + r4
                    rs = slice(r4 * RT, (r4 + 1) * RT)
                    o_ps = cps.tile([128, H], F32, tag="o", bufs=2,
                                    name=f"o_ps{rt}")
                    for p in range(NPAIR):
                        st, sp = (p == 0), (p == NPAIR - 1)
                        nc.tensor.matmul(o_ps[:, 0:512], ctx_ch[:, p, rs],
                                         wo_sb[:, p, 0:512], start=st, stop=sp)
                        nc.tensor.matmul(o_ps[:, 512:768], ctx_ch[:, p, rs],
                                         wo_sb[:, p, 512:768], start=st, stop=sp)
                    o_sb = csb.tile([128, H], BF16, tag="osb", bufs=3,
                                    name=f"o_sb{rt}")
                    nc.vector.tensor_tensor(out=o_sb[:], in0=o_ps[:], in1=bobc[:],
                                            op=ALU.add)
                    nc.sync.dma_start(out=out[rt * RT:(rt + 1) * RT, :], in_=o_sb[:])

    _split_multi_waits(nc)
    return nc


_CACHE = {}
TRACE = False          # set by test harness to capture an NTFF profile
LAST_EXEC_NS = None    # filled on a TRACE run


def _get_nc():
    if "nc" not in _CACHE:
        nc = bass.Bass("TRN2", target_bir_lowering=False, debug=False,
                       num_devices=NCORES)
        _CACHE["nc"] = _build(nc)
    return _CACHE["nc"]


def kernel(hidden_states, Wq, bq, Wk, bk, Wv, bv, Wo, bo, projection_matrix):
    nc = _get_nc()
    BFD = ml_dtypes.bfloat16
    xf = np.asarray(hidden_states, dtype=np.float32).reshape(B * S, H)
    xf = xf.astype(BFD)
    pm = np.asarray(projection_matrix, dtype=np.float32)
    wq_f = np.asarray(Wq, dtype=np.float32)
    wk_f = np.asarray(Wk, dtype=np.float32)
    bq_f = np.asarray(bq, dtype=np.float32)
    bk_f = np.asarray(bk, dtype=np.float32)
    bv_f = np.asarray(bv, dtype=np.float32)
    bo_f = np.asarray(bo, dtype=np.float32)
    # fold the feature projection into the q and k weights (exact in fp32)
    wqp = np.zeros((H, H), np.float32)
    wkp = np.zeros((H, H), np.float32)
    bqp = np.zeros((H,), np.float32)
    bkp = np.zeros((H,), np.float32)
    for h in range(NH):
        cols = slice(h * HD, (h + 1) * HD)
        wqp[:, cols] = wq_f[:, cols] @ pm[h]
        wkp[:, cols] = wk_f[:, cols] @ pm[h]
        bqp[cols] = bq_f[cols] @ pm[h]
        bkp[cols] = bk_f[cols] @ pm[h]
    bqpc = np.ascontiguousarray(bqp.reshape(KT, 128).T)            # [128, KT]
    ebkp = np.ascontiguousarray(
        np.exp(bkp).reshape(NPAIR, 128).transpose(1, 0))           # [128, NPAIR]
    bvbc = np.zeros((128, NPAIR, 130), np.float32)
    for p in range(NPAIR):
        bvbc[:, p, 0:64] = bv_f[2 * p * HD:(2 * p + 1) * HD]
        bvbc[:, p, 65:129] = bv_f[(2 * p + 1) * HD:(2 * p + 2) * HD]
    bobc = np.ascontiguousarray(np.broadcast_to(bo_f, (128, H)))
    sel12 = np.zeros((NH, NPAIR, 128), np.float32)
    for p in range(NPAIR):
        sel12[2 * p, p, 0:64] = 1.0
        sel12[2 * p + 1, p, 64:128] = 1.0
    shared = {
        "wqp": wqp.astype(BFD), "wkp": wkp.astype(BFD),
        "wv": np.asarray(Wv, np.float32).astype(BFD),
        "wo": np.asarray(Wo, np.float32).astype(BFD),
        "bqpc": bqpc, "ebkp": ebkp, "bvbc": bvbc, "bobc": bobc,
        "sel12": sel12.astype(BFD),
    }
    in_maps = [{"xt": np.ascontiguousarray(xf[c * R:(c + 1) * R].T), **shared}
               for c in range(NCORES)]
    res = run_bass_kernel_spmd(nc, in_maps, core_ids=list(range(NCORES)),
                               trace=TRACE)
    if TRACE:
        global LAST_EXEC_NS
        LAST_EXEC_NS = res.exec_time_ns
    outs = [res.results[c]["out"] for c in range(NCORES)]
    return np.concatenate(outs, axis=0).astype(np.float32).reshape(B, S, H)



# revision 4
# speedup vs baseline: 1.0981x; 1.0981x over previous
"""Low-rank (random-feature) attention kernel for Trainium2, 8 NeuronCores — v3.

Sharding: flatten hidden_states to [B*S, H] = [32768, 768] rows; core c owns
4096 contiguous rows (= batch c//2, sequence half c%2).  The per-(batch, head)
kv summary is reduced with two pipelined pairwise AllReduces (~200 KB each).

v3 structural changes vs v2 (425 us):
- Phase 1 row-tile loop reordered (all kp matmuls, then emit_kv(rt-1), then
  all v matmuls) with kp_ps double-buffered and the kv accumulators single-
  buffered: PSUM = 4+2+2 = 8 banks, and the per-tile exp/max drain now has a
  full GEMM of slack -> removes the 1.7 us PE stall per row tile (~55 us).
- bkp is folded into the q-side exp bias on the host (qp' = exp(x Wqp + bqp
  + bkp) scales numerator and denominator identically), and bv rides bo as
  bo' = bo + bv @ Wo (exact up to an O(eps/n) ~ 1e-6 term).  The whole
  post-collective kv fixup chain (ebkp/bvbc scalar_tensor_tensor ops) is
  gone; after the AllReduce only dtype-converting block-diag copies remain,
  so the normalizer matmuls no longer stall ~19 us on DVE.
- bd_kv / ks_bd memsets hoisted to the DMA load phase.
- xT row-tile loads split across the sync and vector DGE queues (halves the
  time to first real matmul).
- bf16 everywhere off-chip (including the output, cast back on host).
"""

import sys

sys.path.insert(0, "/opt/trn_rl_repo")

import contextlib

import ml_dtypes
import numpy as np

import concourse.bass as bass
import concourse.tile as tile
from concourse import mybir
from concourse.bass_utils import run_bass_kernel_spmd

BF16 = mybir.dt.bfloat16
F32 = mybir.dt.float32
AF = mybir.ActivationFunctionType
ALU = mybir.AluOpType
AX = mybir.AxisListType

B, S, H = 4, 8192, 768
NH, HD, M = 12, 64, 64
EPS = 1e-6
NCORES = 8
R = (B * S) // NCORES          # rows per core = 4096
NPAIR = NH // 2                # head pairs = 6
KT = H // 128                  # hidden k-tiles = 6
RT = 128                       # row tile
NRT = R // RT                  # 32 row tiles
CHUNK = 512
NCH = R // CHUNK               # 8 chunks
WARM_MM = 12                   # PE warm-up matmuls during the load phase


def _split_multi_waits(nc):
    """This container's walrus only accepts one semaphore wait per
    instruction; hoist extra waits onto same-engine NoOps placed before."""
    c = 0
    for f in nc.m.functions:
        for bb in f.blocks:
            new_insts = []
            for ins in bb.instructions:
                si = ins.sync_info
                if si is not None and si.on_wait and len(si.on_wait) > 1:
                    waits = list(si.on_wait)
                    for w in waits[:-1]:
                        c += 1
                        new_insts.append(mybir.InstNoOp(
                            name=f"I-waitsplit-{c}", engine=ins.engine,
                            sync_info=mybir.SyncInfo(on_wait=[w], on_update=[])))
                    ins.sync_info = mybir.SyncInfo(
                        on_wait=[waits[-1]], on_update=list(si.on_update))
                new_insts.append(ins)
            bb.instructions[:] = new_insts
    return c


def _bc_free(ap, n):
    """Broadcast an AP along a new innermost (stride-0) free axis of size n."""
    return bass.AP(tensor=ap.tensor, offset=ap.offset, ap=list(ap.ap) + [[0, n]])


def _build(nc):
    xt = nc.dram_tensor("xt", [H, R], BF16, kind="ExternalInput")
    wkp = nc.dram_tensor("wkp", [H, H], BF16, kind="ExternalInput")
    wv = nc.dram_tensor("wv", [H, H], BF16, kind="ExternalInput")
    wqp = nc.dram_tensor("wqp", [H, H], BF16, kind="ExternalInput")
    wo = nc.dram_tensor("wo", [H, H], BF16, kind="ExternalInput")
    bqpc_d = nc.dram_tensor("bqpc", [128, KT], F32, kind="ExternalInput")
    bobc_d = nc.dram_tensor("bobc", [128, H], F32, kind="ExternalInput")
    sel12_d = nc.dram_tensor("sel12", [NH, NPAIR, 128], BF16, kind="ExternalInput")
    out = nc.dram_tensor("out", [R, H], BF16, kind="ExternalOutput")

    with tile.TileContext(nc) as tc, contextlib.ExitStack() as ctx:
        persist = ctx.enter_context(tc.tile_pool(name="persist", bufs=1))
        dram = ctx.enter_context(tc.tile_pool(name="dram", bufs=1, space="DRAM"))

        # ---- PE warm-up fodder: available immediately (no DMA dependency) ----
        junk = persist.tile([128, 512], BF16, tag="junk", name="junk")
        nc.vector.memset(junk[:], 1.0)
        epsc = persist.tile([128, 1], F32, tag="epsc", name="epsc")
        nc.vector.memset(epsc[:], EPS)

        # ---- constants (gpsimd SWDGE queue, small) ----
        bqpc = persist.tile([128, KT], F32, tag="bqpc", name="bqpc")
        nc.gpsimd.dma_start(out=bqpc[:], in_=bqpc_d.ap())
        bobc = persist.tile([128, H], F32, tag="bobc", name="bobc")
        nc.gpsimd.dma_start(out=bobc[:], in_=bobc_d.ap())
        sel12 = persist.tile([NH, NPAIR, 128], BF16, tag="sel12", name="sel12")
        nc.gpsimd.dma_start(out=sel12[:], in_=sel12_d.ap())

        # ---- weights + xT interleaved across the sync and scalar HWDGE rings.
        # Per-ring order is first-needed-first: wkp halves, then x chunk 0
        # (first 4 row tiles), then wv halves (one ~3 us stall at v(0) beats
        # pushing kp(0) back), then the rest ahead of phase-1 consumption.
        wkp_sb = persist.tile([128, KT, H], BF16, tag="wkp", name="wkp_sb")
        wv_sb = persist.tile([128, KT, H], BF16, tag="wv", name="wv_sb")
        wqp_sb = persist.tile([128, KT, H], BF16, tag="wqp", name="wqp_sb")
        wo_sb = persist.tile([128, KT, H], BF16, tag="wo", name="wo_sb")
        xt_sb = persist.tile([128, KT, R], BF16, tag="xt", name="xt_sb")

        def load_w(wsb, wdram, half):
            ks = slice(0, 3) if half == 0 else slice(3, KT)
            eng = nc.sync if half == 0 else nc.scalar
            eng.dma_start(out=wsb[:, ks, :],
                          in_=wdram.ap().rearrange("(k p) n -> p k n", p=128)[:, ks, :])

        def load_x(ch):
            cs = slice(ch * CHUNK, (ch + 1) * CHUNK)
            for kt in range(KT):
                eng = nc.sync if kt % 2 == 0 else nc.scalar
                eng.dma_start(out=xt_sb[:, kt, cs],
                              in_=xt[kt * 128:(kt + 1) * 128, cs])

        load_w(wkp_sb, wkp, 0); load_w(wkp_sb, wkp, 1)
        load_x(0)
        load_w(wv_sb, wv, 0); load_w(wv_sb, wv, 1)
        load_x(1)
        load_w(wqp_sb, wqp, 0); load_w(wqp_sb, wqp, 1)
        load_x(2)
        load_w(wo_sb, wo, 0); load_w(wo_sb, wo, 1)
        for ch in range(3, NCH):
            load_x(ch)

        # ---- persistent result tiles ----
        qpT = [persist.tile([128, R], BF16, tag=f"qpT{p}", name=f"qpT{p}")
               for p in range(NPAIR)]
        kv_acc = [persist.tile([128, NPAIR, 130], F32, tag=f"kvacc{h}",
                               name=f"kv_acc{h}") for h in range(2)]
        kv_fix = persist.tile([128, NPAIR, 130], F32, tag="kvfix", name="kv_fix")
        bd_kv = persist.tile([128, NPAIR, 128], BF16, tag="bdkv", name="bd_kv")
        ks_bd = persist.tile([128, NPAIR, NH], BF16, tag="ksbd", name="ks_bd")
        r_cat = persist.tile([NH, R], BF16, tag="rcat", name="r_cat")
        # block-diag scaffolding zeroed while DVE is otherwise idle
        nc.vector.memset(bd_kv[:], 0.0)
        nc.vector.memset(ks_bd[:], 0.0)

        cc_in = [dram.tile([128, NPAIR * 130], F32, name=f"cc_in{h}") for h in range(2)]
        cc_out = [dram.tile([128, NPAIR * 130], F32, name=f"cc_out{h}") for h in range(2)]

        # ================= Phase 1: k/v pass + kv accumulation =================
        with contextlib.ExitStack() as kctx:
            kvsb = kctx.enter_context(tc.tile_pool(name="kvsb", bufs=2))
            kvps = kctx.enter_context(tc.tile_pool(name="kvps", bufs=1, space="PSUM"))

            # warm-up matmuls share the kp_ps PSUM slot (bufs=2)
            warm_ps = kvps.tile([128, H], F32, tag="kp_ps", bufs=2, name="warm_ps")
            for i in range(WARM_MM):
                nc.tensor.matmul(warm_ps[:, 0:512], junk[:, 0:128], junk[:, :],
                                 start=True, stop=True)

            kv_ps = [None, None]   # [kv_a, kv_b] of current half
            kp_sbs = {}
            v_sbs = {}

            def emit_kv(rt):
                hh, rl = rt // (NRT // 2), rt % (NRT // 2)
                if rl == 0:
                    kv_ps[0] = kvps.tile([128, 3, 130], F32, tag="kv_a",
                                         bufs=1, name=f"kv_a{hh}")
                    kv_ps[1] = kvps.tile([128, 3, 130], F32, tag="kv_b",
                                         bufs=1, name=f"kv_b{hh}")
                kp_sb, v_sb = kp_sbs.pop(rt), v_sbs.pop(rt)
                for p in range(NPAIR):
                    # start=True clears has_written for the WHOLE bank, so only
                    # the first region of each bank may issue it; the other two
                    # pairs overwrite-on-clear at rl==0 via has_written=0.
                    nc.tensor.matmul(
                        kv_ps[p // 3][:, p % 3, :],
                        kp_sb[:, p * 128:(p + 1) * 128],
                        v_sb[:, 2 * p:2 * p + 2, :],
                        start=(rl == 0 and p % 3 == 0),
                        stop=(rl == NRT // 2 - 1),
                        skip_group_check=True)

            def drain_kv(hh):
                nc.vector.tensor_copy(out=kv_acc[hh][:, 0:3, :], in_=kv_ps[0][:])
                nc.vector.tensor_copy(out=kv_acc[hh][:, 3:6, :], in_=kv_ps[1][:])
                nc.sync.dma_start(out=cc_in[hh][:],
                                  in_=kv_acc[hh].rearrange("p a b -> p (a b)"))
                nc.gpsimd.collective_compute(
                    "AllReduce", ALU.add,
                    replica_groups=[[0, 1], [2, 3], [4, 5], [6, 7]],
                    ins=[cc_in[hh].opt()], outs=[cc_out[hh].opt()])

            for rt in range(NRT):
                rs = slice(rt * RT, (rt + 1) * RT)
                kp_ps = kvps.tile([128, H], F32, tag="kp_ps", bufs=2,
                                  name=f"kp_ps{rt}")
                v_ps = kvps.tile([128, H], F32, tag="v_ps", name=f"v_ps{rt}")
                for kt in range(KT):
                    xblk = xt_sb[:, kt, rs]
                    st, sp = (kt == 0), (kt == KT - 1)
                    nc.tensor.matmul(kp_ps[:, 0:512], xblk, wkp_sb[:, kt, 0:512],
                                     start=st, stop=sp)
                    nc.tensor.matmul(kp_ps[:, 512:768], xblk, wkp_sb[:, kt, 512:768],
                                     start=st, stop=sp)
                # kv outer products for the previous row tile sit between the
                # kp and v GEMMs of this one, so the previous tile's exp/scale
                # chain and this tile's kp drain both have a GEMM of slack
                if rt > 0:
                    emit_kv(rt - 1)
                    if rt == NRT // 2:
                        drain_kv(0)
                for kt in range(KT):
                    xblk = xt_sb[:, kt, rs]
                    st, sp = (kt == 0), (kt == KT - 1)
                    nc.tensor.matmul(v_ps[:, 0:512], xblk, wv_sb[:, kt, 0:512],
                                     start=st, stop=sp)
                    nc.tensor.matmul(v_ps[:, 512:768], xblk, wv_sb[:, kt, 512:768],
                                     start=st, stop=sp)
                # postprocess this row tile
                nmx = kvsb.tile([128, NH], F32, tag="nmx", name=f"nmx{rt}")
                nc.vector.tensor_reduce(
                    out=nmx[:], in_=kp_ps.rearrange("p (h m) -> p h m", m=M),
                    op=ALU.max, axis=AX.X, negate=True)
                emax = kvsb.tile([128, NH], F32, tag="emax", name=f"emax{rt}")
                nc.scalar.activation(out=emax[:], in_=nmx[:], func=AF.Exp)
                kp_sb = kvsb.tile([128, H], BF16, tag="kp_sb", name=f"kp_sb{rt}")
                nc.scalar.activation(out=kp_sb[:], in_=kp_ps[:], func=AF.Exp)
                v_sb = kvsb.tile([128, NH, 65], BF16, tag="v_sb", name=f"v_sb{rt}")
                nc.vector.tensor_tensor(
                    out=v_sb[:, :, 0:64],
                    in0=v_ps.rearrange("p (h d) -> p h d", d=HD),
                    in1=_bc_free(emax[:], HD), op=ALU.mult)
                nc.vector.tensor_copy(out=v_sb[:, :, 64:65], in_=emax[:])
                kp_sbs[rt], v_sbs[rt] = kp_sb, v_sb
            emit_kv(NRT - 1)
            drain_kv(1)

        # ---- collective readback + block-diag assembly (copies only) ----
        kv_r = [persist.tile([128, NPAIR, 130], F32, tag=f"kvr{h}", name=f"kv_r{h}")
                for h in range(2)]
        for hh in range(2):
            nc.sync.dma_start(out=kv_r[hh][:],
                              in_=cc_out[hh].rearrange("p (a b) -> p a b", b=130))
        nc.vector.tensor_add(out=kv_fix[:], in0=kv_r[0][:], in1=kv_r[1][:])
        for p in range(NPAIR):
            nc.vector.tensor_copy(out=bd_kv[0:64, p, 0:64], in_=kv_fix[0:64, p, 0:64])
            nc.vector.tensor_copy(out=bd_kv[64:128, p, 64:128],
                                  in_=kv_fix[64:128, p, 65:129])
            nc.vector.tensor_copy(out=ks_bd[0:64, p, 2 * p:2 * p + 1],
                                  in_=kv_fix[0:64, p, 64:65])
            nc.vector.tensor_copy(out=ks_bd[64:128, p, 2 * p + 1:2 * p + 2],
                                  in_=kv_fix[64:128, p, 129:130])

        # ================= Phase 2: q pass, then normalizer =================
        with contextlib.ExitStack() as qctx:
            qsb = qctx.enter_context(tc.tile_pool(name="qsb", bufs=2))
            qps = qctx.enter_context(tc.tile_pool(name="qps", bufs=1, space="PSUM"))
            for ch in range(NCH):
                cs = slice(ch * CHUNK, (ch + 1) * CHUNK)
                for ct in range(KT):
                    qp_ps = qps.tile([128, CHUNK], F32, tag="qp", bufs=3,
                                     name=f"qp_ps{ch}_{ct}")
                    for kt in range(KT):
                        nc.tensor.matmul(qp_ps[:],
                                         wqp_sb[:, kt, ct * 128:(ct + 1) * 128],
                                         xt_sb[:, kt, cs],
                                         start=(kt == 0), stop=(kt == KT - 1))
                    nc.scalar.activation(out=qpT[ct][:, cs], in_=qp_ps[:],
                                         func=AF.Exp, bias=bqpc[:, ct:ct + 1])
            # normalizer: PSUM drains on ACT (frees the np banks quickly, no
            # DVE-order entanglement), then the slow DVE reciprocals pipelined
            # ahead of phase 3's per-chunk consumption
            n_eps_t = []
            for ch in range(NCH):
                cs = slice(ch * CHUNK, (ch + 1) * CHUNK)
                n_ps = qps.tile([NH, CHUNK], F32, tag="np", bufs=2, name=f"n_ps{ch}")
                for p in range(NPAIR):
                    nc.tensor.matmul(n_ps[:], ks_bd[:, p, :], qpT[p][:, cs],
                                     start=(p == 0), stop=(p == NPAIR - 1))
                n_eps = qsb.tile([NH, CHUNK], F32, tag="neps", bufs=4,
                                 name=f"n_eps{ch}")
                nc.scalar.activation(out=n_eps[:], in_=n_ps[:], func=AF.Identity,
                                     bias=epsc[0:NH, :])
                n_eps_t.append(n_eps)
            for ch in range(NCH):
                cs = slice(ch * CHUNK, (ch + 1) * CHUNK)
                r_f32 = qsb.tile([NH, CHUNK], F32, tag="rf32", name=f"r_f32{ch}")
                nc.vector.reciprocal(out=r_f32[:], in_=n_eps_t[ch][:])
                nc.vector.tensor_copy(out=r_cat[:, cs], in_=r_f32[:])

        # ================= Phase 3: ctx + output projection =================
        with contextlib.ExitStack() as cctx:
            csb = cctx.enter_context(tc.tile_pool(name="csb", bufs=2))
            cps = cctx.enter_context(tc.tile_pool(name="cps", bufs=1, space="PSUM"))
            for ch in range(NCH):
                cs = slice(ch * CHUNK, (ch + 1) * CHUNK)
                ctx_ch = csb.tile([128, NPAIR, CHUNK], BF16, tag="ctx",
                                  name=f"ctx{ch}")
                for p in range(NPAIR):
                    rb_ps = cps.tile([128, CHUNK], F32, tag="rb", bufs=2,
                                     name=f"rb_ps{ch}_{p}")
                    nc.tensor.matmul(rb_ps[:], sel12[:, p, :], r_cat[:, cs],
                                     start=True, stop=True)
                    rb_sb = csb.tile([128, CHUNK], F32, tag="rbsb", bufs=2,
                                     name=f"rb_sb{ch}_{p}")
                    nc.scalar.activation(out=rb_sb[:], in_=rb_ps[:], func=AF.Copy)
                    a_ps = cps.tile([128, CHUNK], F32, tag="a", bufs=2,
                                    name=f"a_ps{ch}_{p}")
                    nc.tensor.matmul(a_ps[:], bd_kv[:, p, :], qpT[p][:, cs],
                                     start=True, stop=True)
                    nc.vector.tensor_tensor(out=ctx_ch[:, p, :], in0=a_ps[:],
                                            in1=rb_sb[:], op=ALU.mult)
                for r4 in range(CHUNK // RT):
                    rt = ch * (CHUNK // RT) + r4
                    rs = slice(r4 * RT, (r4 + 1) * RT)
                    o_ps = cps.tile([128, H], F32, tag="o", bufs=2,
                                    name=f"o_ps{rt}")
                    for p in range(NPAIR):
                        st, sp = (p == 0), (p == NPAIR - 1)
                        nc.tensor.matmul(o_ps[:, 0:512], ctx_ch[:, p, rs],
                                         wo_sb[:, p, 0:512], start=st, stop=sp)
                        nc.tensor.matmul(o_ps[:, 512:768], ctx_ch[:, p, rs],
                                         wo_sb[:, p, 512:768], start=st, stop=sp)
                    o_sb = csb.tile([128, H], BF16, tag="osb", bufs=3,
                                    name=f"o_sb{rt}")
                    nc.vector.tensor_tensor(out=o_sb[:], in0=o_ps[:], in1=bobc[:],
                                            op=ALU.add)
                    nc.sync.dma_start(out=out[rt * RT:(rt + 1) * RT, :], in_=o_sb[:])

    _split_multi_waits(nc)
    return nc


_CACHE = {}
TRACE = False          # set by test harness to capture an NTFF profile
LAST_EXEC_NS = None    # filled on a TRACE run


def _get_nc():
    if "nc" not in _CACHE:
        nc = bass.Bass("TRN2", target_bir_lowering=False, debug=False,
                       num_devices=NCORES)
        _CACHE["nc"] = _build(nc)
    return _CACHE["nc"]


def kernel(hidden_states, Wq, bq, Wk, bk, Wv, bv, Wo, bo, projection_matrix):
    nc = _get_nc()
    BFD = ml_dtypes.bfloat16
    xf = np.asarray(hidden_states, dtype=np.float32).reshape(B * S, H)
    xf = xf.astype(BFD)
    pm = np.asarray(projection_matrix, dtype=np.float32)
    wq_f = np.asarray(Wq, dtype=np.float32)
    wk_f = np.asarray(Wk, dtype=np.float32)
    wo_f = np.asarray(Wo, dtype=np.float32)
    bq_f = np.asarray(bq, dtype=np.float32)
    bk_f = np.asarray(bk, dtype=np.float32)
    bv_f = np.asarray(bv, dtype=np.float32)
    bo_f = np.asarray(bo, dtype=np.float32)
    # fold the feature projection into the q and k weights (exact in fp32)
    wqp = np.zeros((H, H), np.float32)
    wkp = np.zeros((H, H), np.float32)
    bqp = np.zeros((H,), np.float32)
    bkp = np.zeros((H,), np.float32)
    for h in range(NH):
        cols = slice(h * HD, (h + 1) * HD)
        wqp[:, cols] = wq_f[:, cols] @ pm[h]
        wkp[:, cols] = wk_f[:, cols] @ pm[h]
        bqp[cols] = bq_f[cols] @ pm[h]
        bkp[cols] = bk_f[cols] @ pm[h]
    # k-side projected bias rides the q-side exp (it scales the ctx numerator
    # and denominator identically); bv rides bo through Wo.
    bqp = bqp + bkp
    bo_f = bo_f + bv_f @ wo_f
    bqpc = np.ascontiguousarray(bqp.reshape(KT, 128).T)            # [128, KT]
    bobc = np.ascontiguousarray(np.broadcast_to(bo_f, (128, H)).copy())
    sel12 = np.zeros((NH, NPAIR, 128), np.float32)
    for p in range(NPAIR):
        sel12[2 * p, p, 0:64] = 1.0
        sel12[2 * p + 1, p, 64:128] = 1.0
    shared = {
        "wqp": wqp.astype(BFD), "wkp": wkp.astype(BFD),
        "wv": np.asarray(Wv, np.float32).astype(BFD),
        "wo": wo_f.astype(BFD),
        "bqpc": bqpc, "bobc": bobc,
        "sel12": sel12.astype(BFD),
    }
    in_maps = [{"xt": np.ascontiguousarray(xf[c * R:(c + 1) * R].T), **shared}
               for c in range(NCORES)]
    res = run_bass_kernel_spmd(nc, in_maps, core_ids=list(range(NCORES)),
                               trace=TRACE)
    if TRACE:
        global LAST_EXEC_NS
        LAST_EXEC_NS = res.exec_time_ns
    outs = [res.results[c]["out"] for c in range(NCORES)]
    return np.concatenate(outs, axis=0).astype(np.float32).reshape(B, S, H)


# revision 7
# speedup vs baseline: 1.1549x; 1.0517x over previous
"""Low-rank (random-feature) attention kernel for Trainium2, 8 NeuronCores — v3.

Sharding: flatten hidden_states to [B*S, H] = [32768, 768] rows; core c owns
4096 contiguous rows (= batch c//2, sequence half c%2).  The per-(batch, head)
kv summary is reduced with two pipelined pairwise AllReduces (~200 KB each).

v3 structural changes vs v2 (425 us):
- Phase 1 row-tile loop reordered (all kp matmuls, then emit_kv(rt-1), then
  all v matmuls) with kp_ps double-buffered and the kv accumulators single-
  buffered: PSUM = 4+2+2 = 8 banks, and the per-tile exp/max drain now has a
  full GEMM of slack -> removes the 1.7 us PE stall per row tile (~55 us).
- bkp is folded into the q-side exp bias on the host (qp' = exp(x Wqp + bqp
  + bkp) scales numerator and denominator identically), and bv rides bo as
  bo' = bo + bv @ Wo (exact up to an O(eps/n) ~ 1e-6 term).  The whole
  post-collective kv fixup chain (ebkp/bvbc scalar_tensor_tensor ops) is
  gone; after the AllReduce only dtype-converting block-diag copies remain,
  so the normalizer matmuls no longer stall ~19 us on DVE.
- bd_kv / ks_bd memsets hoisted to the DMA load phase.
- xT row-tile loads split across the sync and vector DGE queues (halves the
  time to first real matmul).
- bf16 everywhere off-chip (including the output, cast back on host).
"""

import sys

sys.path.insert(0, "/opt/trn_rl_repo")

import contextlib

import ml_dtypes
import numpy as np

import concourse.bass as bass
import concourse.tile as tile
from concourse import mybir
from concourse.bass_utils import run_bass_kernel_spmd

BF16 = mybir.dt.bfloat16
F32 = mybir.dt.float32
AF = mybir.ActivationFunctionType
ALU = mybir.AluOpType
AX = mybir.AxisListType

B, S, H = 4, 8192, 768
NH, HD, M = 12, 64, 64
EPS = 1e-6
NCORES = 8
R = (B * S) // NCORES          # rows per core = 4096
NPAIR = NH // 2                # head pairs = 6
KT = H // 128                  # hidden k-tiles = 6
RT = 128                       # row tile
NRT = R // RT                  # 32 row tiles
CHUNK = 512
NCH = R // CHUNK               # 8 chunks
WARM_MM = 14                   # PE warm-up matmuls during the load phase


def _split_multi_waits(nc):
    """This container's walrus only accepts one semaphore wait per
    instruction; hoist extra waits onto same-engine NoOps placed before."""
    c = 0
    for f in nc.m.functions:
        for bb in f.blocks:
            new_insts = []
            for ins in bb.instructions:
                si = ins.sync_info
                if si is not None and si.on_wait and len(si.on_wait) > 1:
                    waits = list(si.on_wait)
                    for w in waits[:-1]:
                        c += 1
                        new_insts.append(mybir.InstNoOp(
                            name=f"I-waitsplit-{c}", engine=ins.engine,
                            sync_info=mybir.SyncInfo(on_wait=[w], on_update=[])))
                    ins.sync_info = mybir.SyncInfo(
                        on_wait=[waits[-1]], on_update=list(si.on_update))
                new_insts.append(ins)
            bb.instructions[:] = new_insts
    return c


def _bc_free(ap, n):
    """Broadcast an AP along a new innermost (stride-0) free axis of size n."""
    return bass.AP(tensor=ap.tensor, offset=ap.offset, ap=list(ap.ap) + [[0, n]])


def _build(nc):
    xt = nc.dram_tensor("xt", [H, R], BF16, kind="ExternalInput")
    wkp = nc.dram_tensor("wkp", [H, H], BF16, kind="ExternalInput")
    wv = nc.dram_tensor("wv", [H, H], BF16, kind="ExternalInput")
    wqp = nc.dram_tensor("wqp", [H, H], BF16, kind="ExternalInput")
    wo = nc.dram_tensor("wo", [H, H], BF16, kind="ExternalInput")
    bqpc_d = nc.dram_tensor("bqpc", [128, KT], F32, kind="ExternalInput")
    bobc_d = nc.dram_tensor("bobc", [128, H], F32, kind="ExternalInput")
    sel12_d = nc.dram_tensor("sel12", [NH, NPAIR, 128], BF16, kind="ExternalInput")
    out = nc.dram_tensor("out", [R, H], BF16, kind="ExternalOutput")

    with tile.TileContext(nc) as tc, contextlib.ExitStack() as ctx:
        persist = ctx.enter_context(tc.tile_pool(name="persist", bufs=1))
        dram = ctx.enter_context(tc.tile_pool(name="dram", bufs=1, space="DRAM"))

        # ---- PE warm-up fodder: available immediately (no DMA dependency) ----
        junk = persist.tile([128, 512], BF16, tag="junk", name="junk")
        nc.vector.memset(junk[:], 1.0)
        epsc = persist.tile([128, 1], F32, tag="epsc", name="epsc")
        nc.vector.memset(epsc[:], EPS)

        # ---- constants (gpsimd SWDGE queue, small) ----
        bqpc = persist.tile([128, KT], F32, tag="bqpc", name="bqpc")
        nc.gpsimd.dma_start(out=bqpc[:], in_=bqpc_d.ap())
        bobc = persist.tile([128, H], F32, tag="bobc", name="bobc")
        nc.gpsimd.dma_start(out=bobc[:], in_=bobc_d.ap())
        sel12 = persist.tile([NH, NPAIR, 128], BF16, tag="sel12", name="sel12")
        nc.gpsimd.dma_start(out=sel12[:], in_=sel12_d.ap())

        # ---- weights + xT interleaved across the sync and scalar HWDGE rings.
        # Per-ring order is first-needed-first: wkp halves, then x chunk 0
        # (first 4 row tiles), then wv halves (one ~3 us stall at v(0) beats
        # pushing kp(0) back), then the rest ahead of phase-1 consumption.
        wkp_sb = persist.tile([128, KT, H], BF16, tag="wkp", name="wkp_sb")
        wv_sb = persist.tile([128, KT, H], BF16, tag="wv", name="wv_sb")
        wqp_sb = persist.tile([128, KT, H], BF16, tag="wqp", name="wqp_sb")
        wo_sb = persist.tile([128, KT, H], BF16, tag="wo", name="wo_sb")
        xt_sb = persist.tile([128, KT, R], BF16, tag="xt", name="xt_sb")

        def load_w(wsb, wdram, half):
            ks = slice(0, 3) if half == 0 else slice(3, KT)
            eng = nc.sync if half == 0 else nc.scalar
            eng.dma_start(out=wsb[:, ks, :],
                          in_=wdram.ap().rearrange("(k p) n -> p k n", p=128)[:, ks, :])

        def load_x(c0, c1):
            cs = slice(c0, c1)
            for kt in range(KT):
                eng = nc.sync if kt % 2 == 0 else nc.scalar
                eng.dma_start(out=xt_sb[:, kt, cs],
                              in_=xt[kt * 128:(kt + 1) * 128, cs])

        # wkp + the first 4 row tiles gate kp(0); per-kt wv DMAs land just
        # ahead of v(0)'s kt loop; then progressively wider x strips stay
        # ahead of phase-1 consumption; wqp/wo only matter at phases 2/3.
        load_w(wkp_sb, wkp, 0); load_w(wkp_sb, wkp, 1)
        load_x(0, 512)
        for kt in range(KT):
            eng = nc.sync if kt % 2 == 0 else nc.scalar
            eng.dma_start(out=wv_sb[:, kt, :],
                          in_=wv.ap().rearrange("(k p) n -> p k n", p=128)[:, kt, :])
        load_x(512, 1024)
        load_x(1024, 2048)
        load_x(2048, 4096)
        load_w(wqp_sb, wqp, 0); load_w(wqp_sb, wqp, 1)
        load_w(wo_sb, wo, 0); load_w(wo_sb, wo, 1)

        # ---- persistent result tiles ----
        qpT = [persist.tile([128, R], BF16, tag=f"qpT{p}", name=f"qpT{p}")
               for p in range(NPAIR)]
        kv_acc = [persist.tile([128, NPAIR, 130], F32, tag=f"kvacc{h}",
                               name=f"kv_acc{h}") for h in range(2)]
        kv_fix = persist.tile([128, NPAIR, 130], F32, tag="kvfix", name="kv_fix")
        bd_kv = persist.tile([128, NPAIR, 128], BF16, tag="bdkv", name="bd_kv")
        ks_bd = persist.tile([128, NPAIR, NH], BF16, tag="ksbd", name="ks_bd")
        r_cat = persist.tile([NH, R], BF16, tag="rcat", name="r_cat")
        # block-diag scaffolding zeroed while DVE is otherwise idle
        nc.vector.memset(bd_kv[:], 0.0)
        nc.vector.memset(ks_bd[:], 0.0)

        cc_in = [dram.tile([128, NPAIR * 130], F32, name=f"cc_in{h}") for h in range(2)]
        cc_out = [dram.tile([128, NPAIR * 130], F32, name=f"cc_out{h}") for h in range(2)]

        # ================= Phase 1: k/v pass + kv accumulation =================
        with contextlib.ExitStack() as kctx:
            kvsb = kctx.enter_context(tc.tile_pool(name="kvsb", bufs=2))
            kvps = kctx.enter_context(tc.tile_pool(name="kvps", bufs=1, space="PSUM"))

            # warm-up matmuls share the kp_ps PSUM slot (bufs=2)
            warm_ps = kvps.tile([128, H], F32, tag="kp_ps", bufs=2, name="warm_ps")
            for i in range(WARM_MM):
                nc.tensor.matmul(warm_ps[:, 0:512], junk[:, 0:128], junk[:, :],
                                 start=True, stop=True)

            kv_ps = [None, None]   # [kv_a, kv_b] of current half
            kp_sbs = {}
            v_sbs = {}

            def emit_kv(rt):
                hh, rl = rt // (NRT // 2), rt % (NRT // 2)
                if rl == 0:
                    kv_ps[0] = kvps.tile([128, 3, 130], F32, tag="kv_a",
                                         bufs=1, name=f"kv_a{hh}")
                    kv_ps[1] = kvps.tile([128, 3, 130], F32, tag="kv_b",
                                         bufs=1, name=f"kv_b{hh}")
                kp_sb, v_sb = kp_sbs.pop(rt), v_sbs.pop(rt)
                for p in range(NPAIR):
                    # start=True clears has_written for the WHOLE bank, so only
                    # the first region of each bank may issue it; the other two
                    # pairs overwrite-on-clear at rl==0 via has_written=0.
                    nc.tensor.matmul(
                        kv_ps[p // 3][:, p % 3, :],
                        kp_sb[:, p * 128:(p + 1) * 128],
                        v_sb[:, 2 * p:2 * p + 2, :],
                        start=(rl == 0 and p % 3 == 0),
                        stop=(rl == NRT // 2 - 1),
                        skip_group_check=True)

            def drain_kv(hh):
                nc.vector.tensor_copy(out=kv_acc[hh][:, 0:3, :], in_=kv_ps[0][:])
                nc.vector.tensor_copy(out=kv_acc[hh][:, 3:6, :], in_=kv_ps[1][:])
                nc.sync.dma_start(out=cc_in[hh][:],
                                  in_=kv_acc[hh].rearrange("p a b -> p (a b)"))
                nc.gpsimd.collective_compute(
                    "AllReduce", ALU.add,
                    replica_groups=[[0, 1], [2, 3], [4, 5], [6, 7]],
                    ins=[cc_in[hh].opt()], outs=[cc_out[hh].opt()])

            for rt in range(NRT):
                rs = slice(rt * RT, (rt + 1) * RT)
                kp_ps = kvps.tile([128, H], F32, tag="kp_ps", bufs=2,
                                  name=f"kp_ps{rt}")
                v_ps = kvps.tile([128, H], F32, tag="v_ps", name=f"v_ps{rt}")
                for kt in range(KT):
                    xblk = xt_sb[:, kt, rs]
                    st, sp = (kt == 0), (kt == KT - 1)
                    nc.tensor.matmul(kp_ps[:, 0:512], xblk, wkp_sb[:, kt, 0:512],
                                     start=st, stop=sp)
                    nc.tensor.matmul(kp_ps[:, 512:768], xblk, wkp_sb[:, kt, 512:768],
                                     start=st, stop=sp)
                # kv outer products for the previous row tile sit between the
                # kp and v GEMMs of this one, so the previous tile's exp/scale
                # chain and this tile's kp drain both have a GEMM of slack
                if rt > 0:
                    emit_kv(rt - 1)
                    if rt == NRT // 2:
                        drain_kv(0)
                for kt in range(KT):
                    xblk = xt_sb[:, kt, rs]
                    st, sp = (kt == 0), (kt == KT - 1)
                    nc.tensor.matmul(v_ps[:, 0:512], xblk, wv_sb[:, kt, 0:512],
                                     start=st, stop=sp)
                    nc.tensor.matmul(v_ps[:, 512:768], xblk, wv_sb[:, kt, 512:768],
                                     start=st, stop=sp)
                # postprocess this row tile
                nmx = kvsb.tile([128, NH], F32, tag="nmx", name=f"nmx{rt}")
                nc.vector.tensor_reduce(
                    out=nmx[:], in_=kp_ps.rearrange("p (h m) -> p h m", m=M),
                    op=ALU.max, axis=AX.X, negate=True)
                emax = kvsb.tile([128, NH], F32, tag="emax", name=f"emax{rt}")
                nc.scalar.activation(out=emax[:], in_=nmx[:], func=AF.Exp)
                kp_sb = kvsb.tile([128, H], BF16, tag="kp_sb", name=f"kp_sb{rt}")
                nc.scalar.activation(out=kp_sb[:], in_=kp_ps[:], func=AF.Exp)
                v_sb = kvsb.tile([128, NH, 65], BF16, tag="v_sb", name=f"v_sb{rt}")
                nc.vector.tensor_tensor(
                    out=v_sb[:, :, 0:64],
                    in0=v_ps.rearrange("p (h d) -> p h d", d=HD),
                    in1=_bc_free(emax[:], HD), op=ALU.mult)
                nc.vector.tensor_copy(out=v_sb[:, :, 64:65], in_=emax[:])
                kp_sbs[rt], v_sbs[rt] = kp_sb, v_sb
            emit_kv(NRT - 1)
            drain_kv(1)

        # ---- collective readback + block-diag assembly (copies only) ----
        kv_r = [persist.tile([128, NPAIR, 130], F32, tag=f"kvr{h}", name=f"kv_r{h}")
                for h in range(2)]
        for hh in range(2):
            nc.sync.dma_start(out=kv_r[hh][:],
                              in_=cc_out[hh].rearrange("p (a b) -> p a b", b=130))
        nc.vector.tensor_add(out=kv_fix[:], in0=kv_r[0][:], in1=kv_r[1][:])
        for p in range(NPAIR):
            nc.vector.tensor_copy(out=bd_kv[0:64, p, 0:64], in_=kv_fix[0:64, p, 0:64])
            nc.vector.tensor_copy(out=bd_kv[64:128, p, 64:128],
                                  in_=kv_fix[64:128, p, 65:129])
            nc.vector.tensor_copy(out=ks_bd[0:64, p, 2 * p:2 * p + 1],
                                  in_=kv_fix[0:64, p, 64:65])
            nc.vector.tensor_copy(out=ks_bd[64:128, p, 2 * p + 1:2 * p + 2],
                                  in_=kv_fix[64:128, p, 129:130])

        # ================= Phase 2: q pass, then normalizer =================
        with contextlib.ExitStack() as qctx:
            qsb = qctx.enter_context(tc.tile_pool(name="qsb", bufs=2))
            qps = qctx.enter_context(tc.tile_pool(name="qps", bufs=1, space="PSUM"))

            def norm_ch(ch):
                # n is O(1000) (positive exp features dotted with a 32k-row
                # sum), so 1/n == 1/(n+eps) to ~1e-9 and the DVE reciprocal
                # (3.3 us each — the slow op here) reads PSUM directly.
                cs = slice(ch * CHUNK, (ch + 1) * CHUNK)
                n_ps = qps.tile([NH, CHUNK], F32, tag="np", bufs=3, name=f"n_ps{ch}")
                for p in range(NPAIR):
                    nc.tensor.matmul(n_ps[:], ks_bd[:, p, :], qpT[p][:, cs],
                                     start=(p == 0), stop=(p == NPAIR - 1))
                r_f32 = qsb.tile([NH, CHUNK], F32, tag="rf32", name=f"r_f32{ch}")
                nc.vector.reciprocal(out=r_f32[:], in_=n_ps[:])
                nc.vector.tensor_copy(out=r_cat[:, cs], in_=r_f32[:])

            for ch in range(NCH):
                cs = slice(ch * CHUNK, (ch + 1) * CHUNK)
                for ct in range(KT):
                    qp_ps = qps.tile([128, CHUNK], F32, tag="qp", bufs=3,
                                     name=f"qp_ps{ch}_{ct}")
                    for kt in range(KT):
                        nc.tensor.matmul(qp_ps[:],
                                         wqp_sb[:, kt, ct * 128:(ct + 1) * 128],
                                         xt_sb[:, kt, cs],
                                         start=(kt == 0), stop=(kt == KT - 1))
                    nc.scalar.activation(out=qpT[ct][:, cs], in_=qp_ps[:],
                                         func=AF.Exp, bias=bqpc[:, ct:ct + 1])
                # normalizer chunks ride along once ks_bd (collective) is
                # ready, paced so the DVE reciprocals keep up
                if ch >= 4:
                    norm_ch(ch - 4)
            for ch in range(NCH - 4, NCH):
                norm_ch(ch)

        # ================= Phase 3: ctx + output projection =================
        with contextlib.ExitStack() as cctx:
            csb = cctx.enter_context(tc.tile_pool(name="csb", bufs=2))
            cps = cctx.enter_context(tc.tile_pool(name="cps", bufs=1, space="PSUM"))
            for ch in range(NCH):
                cs = slice(ch * CHUNK, (ch + 1) * CHUNK)
                ctx_ch = csb.tile([128, NPAIR, CHUNK], BF16, tag="ctx",
                                  name=f"ctx{ch}")
                for p in range(NPAIR):
                    rb_ps = cps.tile([128, CHUNK], F32, tag="rb", bufs=2,
                                     name=f"rb_ps{ch}_{p}")
                    nc.tensor.matmul(rb_ps[:], sel12[:, p, :], r_cat[:, cs],
                                     start=True, stop=True)
                    rb_sb = csb.tile([128, CHUNK], F32, tag="rbsb", bufs=2,
                                     name=f"rb_sb{ch}_{p}")
                    nc.scalar.activation(out=rb_sb[:], in_=rb_ps[:], func=AF.Copy)
                    a_ps = cps.tile([128, CHUNK], F32, tag="a", bufs=2,
                                    name=f"a_ps{ch}_{p}")
                    nc.tensor.matmul(a_ps[:], bd_kv[:, p, :], qpT[p][:, cs],
                                     start=True, stop=True)
                    nc.vector.tensor_tensor(out=ctx_ch[:, p, :], in0=a_ps[:],
                                            in1=rb_sb[:], op=ALU.mult)
                for r4 in range(CHUNK // RT):
                    rt = ch * (CHUNK // RT) + r4
                    rs = slice(r4 * RT, (r4 + 1) * RT)
                    o_ps = cps.tile([128, H], F32, tag="o", bufs=2,
                                    name=f"o_ps{rt}")
                    for p in range(NPAIR):
                        st, sp = (p == 0), (p == NPAIR - 1)
                        nc.tensor.matmul(o_ps[:, 0:512], ctx_ch[:, p, rs],
                                         wo_sb[:, p, 0:512], start=st, stop=sp)
                        nc.tensor.matmul(o_ps[:, 512:768], ctx_ch[:, p, rs],
                                         wo_sb[:, p, 512:768], start=st, stop=sp)
                    o_sb = csb.tile([128, H], BF16, tag="osb", bufs=3,
                                    name=f"o_sb{rt}")
                    nc.vector.tensor_tensor(out=o_sb[:], in0=o_ps[:], in1=bobc[:],
                                            op=ALU.add)
                    nc.sync.dma_start(out=out[rt * RT:(rt + 1) * RT, :], in_=o_sb[:])

    _split_multi_waits(nc)
    return nc


_CACHE = {}
TRACE = False          # set by test harness to capture an NTFF profile
LAST_EXEC_NS = None    # filled on a TRACE run


def _get_nc():
    if "nc" not in _CACHE:
        nc = bass.Bass("TRN2", target_bir_lowering=False, debug=False,
                       num_devices=NCORES)
        _CACHE["nc"] = _build(nc)
    return _CACHE["nc"]


def kernel(hidden_states, Wq, bq, Wk, bk, Wv, bv, Wo, bo, projection_matrix):
    nc = _get_nc()
    BFD = ml_dtypes.bfloat16
    xf = np.asarray(hidden_states, dtype=np.float32).reshape(B * S, H)
    xf = xf.astype(BFD)
    pm = np.asarray(projection_matrix, dtype=np.float32)
    wq_f = np.asarray(Wq, dtype=np.float32)
    wk_f = np.asarray(Wk, dtype=np.float32)
    wo_f = np.asarray(Wo, dtype=np.float32)
    bq_f = np.asarray(bq, dtype=np.float32)
    bk_f = np.asarray(bk, dtype=np.float32)
    bv_f = np.asarray(bv, dtype=np.float32)
    bo_f = np.asarray(bo, dtype=np.float32)
    # fold the feature projection into the q and k weights (exact in fp32)
    wqp = np.zeros((H, H), np.float32)
    wkp = np.zeros((H, H), np.float32)
    bqp = np.zeros((H,), np.float32)
    bkp = np.zeros((H,), np.float32)
    for h in range(NH):
        cols = slice(h * HD, (h + 1) * HD)
        wqp[:, cols] = wq_f[:, cols] @ pm[h]
        wkp[:, cols] = wk_f[:, cols] @ pm[h]
        bqp[cols] = bq_f[cols] @ pm[h]
        bkp[cols] = bk_f[cols] @ pm[h]
    # k-side projected bias rides the q-side exp (it scales the ctx numerator
    # and denominator identically); bv rides bo through Wo.
    bqp = bqp + bkp
    bo_f = bo_f + bv_f @ wo_f
    bqpc = np.ascontiguousarray(bqp.reshape(KT, 128).T)            # [128, KT]
    bobc = np.ascontiguousarray(np.broadcast_to(bo_f, (128, H)).copy())
    sel12 = np.zeros((NH, NPAIR, 128), np.float32)
    for p in range(NPAIR):
        sel12[2 * p, p, 0:64] = 1.0
        sel12[2 * p + 1, p, 64:128] = 1.0
    shared = {
        "wqp": wqp.astype(BFD), "wkp": wkp.astype(BFD),
        "wv": np.asarray(Wv, np.float32).astype(BFD),
        "wo": wo_f.astype(BFD),
        "bqpc": bqpc, "bobc": bobc,
        "sel12": sel12.astype(BFD),
    }
    in_maps = [{"xt": np.ascontiguousarray(xf[c * R:(c + 1) * R].T), **shared}
               for c in range(NCORES)]
    res = run_bass_kernel_spmd(nc, in_maps, core_ids=list(range(NCORES)),
                               trace=TRACE)
    if TRACE:
        global LAST_EXEC_NS
        LAST_EXEC_NS = res.exec_time_ns
    outs = [res.results[c]["out"] for c in range(NCORES)]
    return np.concatenate(outs, axis=0).astype(np.float32).reshape(B, S, H)
